# revision 13
# baseline (speedup 1.0000x reference)
"""Trainium2 Bass kernel for nn_Encoder (point-cloud encoder with segment-mean).

Strategy: data-parallel over clouds across N_CORES_RUN NeuronCores. Each core
runs a feature-major fused pipeline: point MLP (f16 matmuls on PE, bias+
LeakyReLU evictions on ScalarE), per-segment sums (strided reduce on VectorE);
the segment mean is taken BEFORE the (linear) final point layer, so the last
point layer and the latent MLP run on B_C clouds instead of N_C points.

Dispatch-overhead note: per-execute host cost scales with the number of tensor
bindings (cores x tensors), so ALL inputs are packed into ONE f16 blob per
core (xt region + a [128, WALL] weight/aux area addressed via AP views of the
flat DRAM tensor), and there is one output tensor.

Reference-semantics note: the oracle's `idx` is produced with int32 overflow,
which makes its searchsorted assign every point segment id 4096 - all points
are dropped by segment_sum and the oracle latent input is exactly zero. The
kernel reproduces the oracle's semantics via two host-computed scalars applied
on-chip as data:
  factor - multiplies the on-chip per-cloud h2 segment sums (0 when the oracle
           drops all points; 1/256 for the uniform contiguous layout)
  c3     - scales the final point-layer bias contribution, added back via a
           rank-1 accumulating matmul
so the heavy device pipeline is unchanged while the output matches the oracle.
"""
import numpy as np
import concourse.bass as bass
import concourse.mybir as mybir
from concourse.tile import TileContext
from concourse.bass_utils import run_bass_kernel_spmd

F32 = mybir.dt.float32
F16 = mybir.dt.float16

N_TOTAL = 1_048_576
B = 4096
SEG = N_TOTAL // B            # 256 points per (uniform) cloud
CHUNK = 1024                  # points processed per loop iteration
HALF = CHUNK // 2
CC = 512                      # tail cloud-chunk (1 PSUM bank of f32)


def _set_cores(n):
    """Derive per-core geometry; default is set at module bottom."""
    global N_CORES_RUN, N_C, B_C, N_ITER, N_CC, XTSZ, BLOB, _NC_CACHE
    N_CORES_RUN = n
    N_C = N_TOTAL // n        # points per core
    B_C = B // n              # clouds per core
    N_ITER = N_C // CHUNK
    N_CC = B_C // CC if B_C >= CC else 1
    XTSZ = 10 * (N_C // 2)
    BLOB = XTSZ + 128 * WALL
    _NC_CACHE = None

# ---- weight/aux area column map (f16), [128, WALL] ----
WHC = 0              # cols [0:192): L0/L1 weights (dual-row-group layout)
W2C = 192            # cols [192:448): pw2.T as lo|hi 128-col halves
WTC = 448            # 20 tail blocks x 128 cols: w3,lw0,lw1,mw,vw x (00,10,01,11)
B3C = WTC + 20 * 128         # row 0: pb3 lo [B3C:B3C+128), hi [B3C+128:B3C+256)
BPC = B3C + 256              # cols [BPC:BPC+16): biases
FTC = BPC + 16               # cols [FTC:FTC+2): factor, c3 (replicated rows)
C3C = FTC + 2                # row 0, cols [C3C:C3C+CC): c3 row for rank-1 mm
WALL = C3C + CC
_TAIL_ORDER = ("w3", "lw0", "lw1", "mw", "vw")
# bias column map inside BPC block
_BIAS_COL = {"b1": 1, "b2lo": 2, "b2hi": 3, "lb0lo": 6, "lb0hi": 7,
             "lb1lo": 8, "lb1hi": 9, "mblo": 10, "mbhi": 11,
             "vblo": 12, "vbhi": 13}

_set_cores(4)


def _tail_off(pref, k, m):
    i = _TAIL_ORDER.index(pref)
    return WTC + (i * 4 + k * 1 + m * 2) * 128


def _split_multi_waits(nc):
    """This walrus build supports only one sync-wait per lowered instruction;
    split extra waits into preceding single-wait EventSemaphore NOPs."""
    ctr = 0
    for f in nc.m.functions:
        for blk in f.blocks:
            out = []
            changed = False
            for inst in blk.instructions:
                si = inst.sync_info
                waits = list(si.on_wait) if si is not None else []
                if len(waits) > 1:
                    for w in waits[:-1]:
                        ctr += 1
                        ev = mybir.InstEventSemaphore(
                            name=f"antwaitsplit-{ctr}", ins=[], outs=[],
                            sync_info=mybir.SyncInfo(on_wait=[w], on_update=[]),
                        )
                        ev.engine = inst.engine
                        out.append(ev)
                    inst.sync_info = mybir.SyncInfo(
                        on_wait=[waits[-1]], on_update=list(si.on_update))
                    changed = True
                out.append(inst)
            if changed:
                blk.instructions = out
    return ctr


def build_nc():
    nc = bass.Bass(enable_partition_id=False)
    LR = mybir.ActivationFunctionType.Lrelu
    COPY = mybir.ActivationFunctionType.Copy
    IDENT = mybir.ActivationFunctionType.Identity
    AX = mybir.AxisListType.X
    MULT = mybir.AluOpType.mult
    MAX = mybir.AluOpType.max

    blob_d = nc.dram_tensor("blob", [1, BLOB], F16, kind="ExternalInput")
    o_d = nc.dram_tensor("outp", [128, 4 * B_C], F32, kind="ExternalOutput")
    xtv = blob_d[0, 0:XTSZ].rearrange("(a b) -> a b", a=10)
    wv = blob_d[0, XTSZ:XTSZ + 128 * WALL].rearrange("(p c) -> p c", p=128)

    with TileContext(nc) as tc:
        with (
            tc.tile_pool(name="wp", bufs=1) as wp,
            tc.tile_pool(name="xp", bufs=6) as xp,
            tc.tile_pool(name="ap", bufs=3) as ap,
            tc.tile_pool(name="sp", bufs=1) as spp,
        ):
            wt = wp.tile([128, WALL], F16)
            nc.sync.dma_start(wt[:, :], wv[:, :])
            ftf = wp.tile([128, 2], F32)
            nc.vector.tensor_copy(ftf[:, :], wt[:, FTC:FTC + 2])

            def BIAS(name):
                c = BPC + _BIAS_COL[name]
                return wt[:, c:c + 1]

            segsum_lo = spp.tile([128, B_C], F32)
            segsum_hi = spp.tile([128, B_C], F32)

            # 512-col PSUM tiles so every tag double-buffers within 8 banks:
            # p0 2x1 + p1 2x1 + p2a 2x1 + p2b 2x1 = 8 banks
            with tc.tile_pool(name="psA", bufs=2, space="PSUM") as psA:
                for i in range(N_ITER):
                    # packed x: chunk-A features+ones at partitions 0:5,
                    # chunk-B at partitions 32:37
                    xt_i = xp.tile([64, HALF], F16, name="xt_i")
                    nc.sync.dma_start(xt_i[0:5, :], xtv[0:5, i * HALF:(i + 1) * HALF])
                    nc.sync.dma_start(xt_i[32:37, :], xtv[5:10, i * HALF:(i + 1) * HALF])

                    # L0 (bias folded in via the ones row): two concurrent
                    # row-group matmuls -> p0 holds lrelu input y for A|B packed
                    p0 = psA.tile([128, HALF], F32, name="p0", tag="p0")
                    nc.tensor.matmul(p0[0:64, :], wt[0:5, WHC:WHC + 64], xt_i[0:5, :],
                                     start=True, stop=True)
                    nc.tensor.matmul(p0[64:128, :], wt[32:37, WHC:WHC + 64],
                                     xt_i[32:37, :],
                                     start=True, stop=True, tile_position=(32, 64))
                    # lrelu(y) = max(0.01*y, y) on VectorE (2 ops, no ACT)
                    t0 = ap.tile([128, HALF], F32, name="t0", tag="t0")
                    nc.vector.tensor_scalar_mul(t0[:, :], p0[:, :], 0.01)
                    u0 = ap.tile([128, HALF], F16, name="u0", tag="u0")
                    nc.vector.tensor_tensor(u0[:, :], t0[:, :], p0[:, :], MAX)

                    h2lo = ap.tile([128, CHUNK], F16, name="h2lo", tag="h2lo")
                    h2hi = ap.tile([128, CHUNK], F16, name="h2hi", tag="h2hi")
                    # chunk halves A (u0[0:64]) and B (u0[64:128]) flow through
                    # L1 -> L2 in 512-col tiles, pipelined across PSUM tags
                    for q, (r0, tp) in enumerate(((0, None), (64, (64, 0)))):
                        qs = slice(q * HALF, (q + 1) * HALF)
                        p1 = psA.tile([128, HALF], F32, name=f"p1_{q}", tag="p1")
                        nc.tensor.matmul(p1[:, :], wt[r0:r0 + 64, WHC + 64:WHC + 192],
                                         u0[r0:r0 + 64, :], start=True, stop=True,
                                         **({"tile_position": tp} if tp else {}))
                        h1a = ap.tile([128, HALF], F16, name=f"h1a_{q}", tag="h1a")
                        nc.scalar.activation(h1a[:, :], p1[:, :], LR,
                                             bias=BIAS("b1"), alpha=0.01)
                        p2a = psA.tile([128, HALF], F32, name=f"p2a_{q}", tag="p2a")
                        p2b = psA.tile([128, HALF], F32, name=f"p2b_{q}", tag="p2b")
                        nc.tensor.matmul(p2a[:, :], wt[:, W2C:W2C + 128],
                                         h1a[:, :], start=True, stop=True)
                        nc.tensor.matmul(p2b[:, :], wt[:, W2C + 128:W2C + 256],
                                         h1a[:, :], start=True, stop=True)
                        nc.scalar.activation(h2lo[:, qs], p2a[:, :], LR,
                                             bias=BIAS("b2lo"), alpha=0.01)
                        nc.scalar.activation(h2hi[:, qs], p2b[:, :], LR,
                                             bias=BIAS("b2hi"), alpha=0.01)

                    g = CHUNK // SEG
                    nc.vector.reduce_sum(
                        segsum_lo[:, i * g:(i + 1) * g],
                        h2lo[:, :].rearrange("p (g s) -> p g s", s=SEG), axis=AX)
                    nc.vector.reduce_sum(
                        segsum_hi[:, i * g:(i + 1) * g],
                        h2hi[:, :].rearrange("p (g s) -> p g s", s=SEG), axis=AX)

            # ---- tail: scaled mean -> L3 (+bias*c3) -> latent MLP -> outputs ----
            outt = spp.tile([128, 4 * B_C], F32)
            sc_lo = spp.tile([128, B_C], F16)
            sc_hi = spp.tile([128, B_C], F16)
            nc.vector.tensor_scalar(sc_lo[:, :], segsum_lo[:, :], ftf[:, 0:1],
                                    None, op0=MULT)
            nc.vector.tensor_scalar(sc_hi[:, :], segsum_hi[:, :], ftf[:, 0:1],
                                    None, op0=MULT)
            c3row = wt[0:1, C3C:C3C + CC]

            with tc.tile_pool(name="psB", bufs=4, space="PSUM") as psB:
                for cc in range(N_CC):
                    cs = slice(cc * CC, (cc + 1) * CC)

                    def layer(pref, rhs_lo, rhs_hi, bias_lo, bias_hi, func,
                              out_lo=None, out_hi=None, rank1_bias=False):
                        plo = psB.tile([128, CC], F32, name=f"{pref}_plo{cc}",
                                       tag="pt")
                        phi = psB.tile([128, CC], F32, name=f"{pref}_phi{cc}",
                                       tag="pt")
                        for p, m in ((plo, 0), (phi, 1)):
                            nc.tensor.matmul(
                                p[:, :], wt[:, _tail_off(pref, 0, m):
                                            _tail_off(pref, 0, m) + 128],
                                rhs_lo, start=True, stop=False)
                            nc.tensor.matmul(
                                p[:, :], wt[:, _tail_off(pref, 1, m):
                                            _tail_off(pref, 1, m) + 128],
                                rhs_hi, start=False, stop=not rank1_bias)
                            if rank1_bias:
                                b3 = wt[0:1, B3C + m * 128:B3C + (m + 1) * 128]
                                nc.tensor.matmul(p[:, :], b3, c3row,
                                                 start=False, stop=True)
                        if out_lo is None:
                            out_lo = ap.tile([128, CC], F16, name=f"{pref}_olo{cc}",
                                             tag=f"{pref}_olo")
                            out_hi = ap.tile([128, CC], F16, name=f"{pref}_ohi{cc}",
                                             tag=f"{pref}_ohi")
                        if func is COPY:
                            nc.scalar.activation(out_lo, plo[:, :], func)
                            nc.scalar.activation(out_hi, phi[:, :], func)
                        else:
                            nc.scalar.activation(out_lo, plo[:, :], func,
                                                 bias=BIAS(bias_lo), alpha=0.01)
                            nc.scalar.activation(out_hi, phi[:, :], func,
                                                 bias=BIAS(bias_hi), alpha=0.01)
                        return out_lo, out_hi

                    m3 = layer("w3", sc_lo[:, cs], sc_hi[:, cs], None, None,
                               COPY, rank1_bias=True)
                    l0 = layer("lw0", m3[0][:, :], m3[1][:, :], "lb0lo", "lb0hi", LR)
                    l1 = layer("lw1", l0[0][:, :], l0[1][:, :], "lb1lo", "lb1hi", LR)
                    layer("mw", l1[0][:, :], l1[1][:, :], "mblo", "mbhi", IDENT,
                          out_lo=outt[:, 0 * B_C + cc * CC:0 * B_C + (cc + 1) * CC],
                          out_hi=outt[:, 1 * B_C + cc * CC:1 * B_C + (cc + 1) * CC])
                    layer("vw", l1[0][:, :], l1[1][:, :], "vblo", "vbhi", IDENT,
                          out_lo=outt[:, 2 * B_C + cc * CC:2 * B_C + (cc + 1) * CC],
                          out_hi=outt[:, 3 * B_C + cc * CC:3 * B_C + (cc + 1) * CC])
            nc.sync.dma_start(o_d[:, :], outt[:, :])

    _split_multi_waits(nc)
    return nc


_NC_CACHE = None


def _get_nc():
    global _NC_CACHE
    if _NC_CACHE is None:
        _NC_CACHE = build_nc()
    return _NC_CACHE


def _make_warea(pw0, pb0, pw1, pw2, pw3, pb3, lw0, lw1, mw, vw,
                pb1, pb2, lb0, lb1, mb, vb, factor, c3):
    wa = np.zeros((128, WALL), np.float32)
    w0b = np.concatenate([pw0, pb0[:, None]], axis=1).T      # [5, 64]
    wa[0:5, WHC:WHC + 64] = w0b
    wa[32:37, WHC:WHC + 64] = w0b
    wa[0:64, WHC + 64:WHC + 192] = pw1.T
    wa[64:128, WHC + 64:WHC + 192] = pw1.T
    wa[:, W2C:W2C + 256] = pw2.T
    for i, wm in enumerate((pw3.T, lw0.T, lw1.T, mw.T, vw.T)):
        for k in (0, 1):
            for m in (0, 1):
                off = WTC + (i * 4 + k * 1 + m * 2) * 128
                wa[:, off:off + 128] = wm[k * 128:(k + 1) * 128,
                                          m * 128:(m + 1) * 128]
    wa[0, B3C:B3C + 128] = pb3[0:128]
    wa[0, B3C + 128:B3C + 256] = pb3[128:256]
    wa[0:64, BPC + 0] = pb0
    wa[64:128, BPC + 0] = pb0
    wa[:, BPC + 1] = pb1
    for col, vec in zip(("b2lo", "lb0lo", "lb1lo", "mblo", "vblo"),
                        (pb2, lb0, lb1, mb, vb)):
        wa[:, BPC + _BIAS_COL[col]] = vec[0:128]
        wa[:, BPC + _BIAS_COL[col] + 1] = vec[128:256]
    wa[:, FTC] = factor
    wa[:, FTC + 1] = c3
    wa[0, C3C:C3C + CC] = c3
    return wa.astype(np.float16)


def _build_in_maps(points, factor, c3, pw0, pb0, pw1, pb1, pw2, pb2, pw3, pb3,
                   lw0, lb0, lw1, lb1, mw, mb, vw, vb):
    """Per-core single-blob inputs: [xt packed | weight area] flattened f16."""
    xt = points.T.astype(np.float32)                  # [4, N_TOTAL]
    wa_flat = _make_warea(pw0, pb0, pw1, pw2, pw3, pb3, lw0, lw1, mw, vw,
                          pb1, pb2, lb0, lb1, mb, vb, factor, c3).reshape(-1)
    in_maps = []
    for c in range(N_CORES_RUN):
        xs = xt[:, c * N_C:(c + 1) * N_C].reshape(4, N_ITER, 2, HALF)
        xp5 = np.ones((10, N_C // 2), np.float32)
        xp5[0:4] = xs[:, :, 0, :].reshape(4, -1)
        xp5[5:9] = xs[:, :, 1, :].reshape(4, -1)
        blob = np.empty((1, BLOB), np.float16)
        blob[0, 0:XTSZ] = xp5.reshape(-1).astype(np.float16)
        blob[0, XTSZ:] = wa_flat
        in_maps.append({"blob": blob})
    return in_maps


def _unpack_outputs(results):
    mu = np.empty((B, 256), np.float32)
    lv = np.empty((B, 256), np.float32)
    for c in range(N_CORES_RUN):
        o = results[c]["outp"]
        sl = slice(c * B_C, (c + 1) * B_C)
        mu[sl, 0:128] = o[:, 0:B_C].T
        mu[sl, 128:256] = o[:, B_C:2 * B_C].T
        lv[sl, 0:128] = o[:, 2 * B_C:3 * B_C].T
        lv[sl, 128:256] = o[:, 3 * B_C:4 * B_C].T
    return mu, lv


def _reference_numpy(points, idx, pw0, pb0, pw1, pb1, pw2, pb2, pw3, pb3,
                     lw0, lb0, lw1, lb1, mw, mb, vw, vb):
    """Exact-semantics fallback for segment layouts the device path doesn't
    model (never taken for the staged problem)."""
    def lrelu(x):
        return np.where(x > 0, x, np.float32(0.01) * x)
    h = lrelu(points @ pw0.T + pb0)
    h = lrelu(h @ pw1.T + pb1)
    h = lrelu(h @ pw2.T + pb2)
    h = h @ pw3.T + pb3
    n, b = h.shape[0], idx.shape[0]
    seg = np.searchsorted(idx, np.arange(n).astype(idx.dtype), side="right")
    valid = (seg >= 0) & (seg < b)
    sums = np.zeros((b, h.shape[1]), np.float32)
    np.add.at(sums, seg[valid], h[valid])
    starts = np.concatenate([idx[:1] * 0, idx[:-1]])
    counts = (idx - starts).astype(np.float32)
    with np.errstate(all="ignore"):
        latent = sums / counts[:, None]
    latent = lrelu(latent @ lw0.T + lb0)
    latent = lrelu(latent @ lw1.T + lb1)
    return latent @ mw.T + mb, latent @ vw.T + vb


def kernel(points, idx, pw0, pb0, pw1, pb1, pw2, pb2, pw3, pb3,
           lw0, lb0, lw1, lb1, mw, mb, vw, vb):
    points = np.asarray(points, np.float32)
    idx = np.asarray(idx)
    (pw0, pb0, pw1, pb1, pw2, pb2, pw3, pb3,
     lw0, lb0, lw1, lb1, mw, mb, vw, vb) = [
        np.asarray(a, np.float32) for a in
        (pw0, pb0, pw1, pb1, pw2, pb2, pw3, pb3, lw0, lb0, lw1, lb1, mw, mb, vw, vb)]

    n, b = points.shape[0], idx.shape[0]
    # replicate the oracle's segment assignment (including any idx overflow)
    seg = np.searchsorted(idx, np.arange(n).astype(idx.dtype), side="right")
    starts = np.concatenate([idx[:1] * 0, idx[:-1]])
    counts = (idx - starts).astype(np.float32)
    uniform_layout = (n == N_TOTAL and b == B and
                      np.array_equal(seg, np.arange(n) // SEG) and
                      np.all(counts == SEG))
    all_dropped = bool(np.all((seg < 0) | (seg >= b))) and n == N_TOTAL and b == B

    if uniform_layout:
        factor = np.float32(1.0 / SEG)
        c3 = np.float32(1.0)
    elif all_dropped:
        factor = np.float32(0.0)
        c3 = np.float32(0.0)
    else:
        return _reference_numpy(points, idx, pw0, pb0, pw1, pb1, pw2, pb2, pw3,
                                pb3, lw0, lb0, lw1, lb1, mw, mb, vw, vb)

    in_maps = _build_in_maps(points, factor, c3, pw0, pb0, pw1, pb1, pw2, pb2,
                             pw3, pb3, lw0, lb0, lw1, lb1, mw, mb, vw, vb)
    nc = _get_nc()
    res = run_bass_kernel_spmd(nc, in_maps, core_ids=list(range(N_CORES_RUN)))
    return _unpack_outputs(res.results)
